# revision 22
# baseline (speedup 1.0000x reference)
"""MASS variational distribution head: MOG class log-likelihood + log_softmax.

Takes FULL inputs, returns FULL output [B, C]. Internally class-sharded
across 8 NeuronCores (13 padded classes per core), single NEFF, one
AllGather of the per-class log-probs before the final log_softmax.

Math per (class c, component k), all on device:
  A = L^{-1}  via truncated doubling A ~= (I+X)(I+X^2), X = I - L
  M = A^T A,  v = M mu,  s = mu^T v,  logdet = sum log|diag L|
  comp(x) = -0.5 x^T M x + v.x - 0.5 s - 0.5 D log(2pi) - logdet + logmix
  class_lp = logsumexp_k comp ; out = log_softmax_c class_lp

comp is evaluated as one feature matmul S = W^T.T @ F over 4224 features
[x_i x_j (4096, -0.5 folded into one x) | x (64) | 1 | 1 | pad], W bf16.
A global SHIFT is folded into the constant so both logsumexps reduce to
plain exp (ScalarE) + ones-matmul sums (TensorE) without max-subtraction.
"""
import functools
import numpy as np

B, D, C, K = 2048, 64, 100, 8
NCORES = 8
CP = 104                 # padded class count (8 * 13)
CC = CP // NCORES        # classes per core = 13
CKC = CC * K             # ck per core = 104
NPAIR = CKC // 2         # 52
NQ = NPAIR // 4          # 13 four-pair batches
NT = D * D // 128        # 32 quad feature chunks
NB = B // 512            # 4 psum column blocks
SHIFT = 100.0
LOG2PI = 1.8378770664093453
PAD_MU = 1.0e3


@functools.lru_cache(maxsize=2)
def _build_nc(debug=False):
    import concourse.bacc as bacc
    import concourse.mybir as mybir
    import concourse.tile as tile

    dt = mybir.dt
    AF = mybir.ActivationFunctionType
    nc = bacc.Bacc("TRN2", target_bir_lowering=False, debug=False,
                   num_devices=NCORES)

    Lp = nc.dram_tensor("Lp", [128, NPAIR * 128], dt.bfloat16, kind="ExternalInput")
    LpT = nc.dram_tensor("LpT", [128, NPAIR * 128], dt.bfloat16, kind="ExternalInput")
    xt = nc.dram_tensor("xt", [D, B], dt.bfloat16, kind="ExternalInput")
    muT = nc.dram_tensor("muT", [D, CKC], dt.float32, kind="ExternalInput")
    mixc = nc.dram_tensor("mixc", [CC, K], dt.float32, kind="ExternalInput")
    eye4b = nc.dram_tensor("eye4b", [128, 512], dt.bfloat16, kind="ExternalInput")
    eye1b = nc.dram_tensor("eye1b", [128, 128], dt.bfloat16, kind="ExternalInput")
    oneskt = nc.dram_tensor("oneskt", [CKC, CC], dt.bfloat16, kind="ExternalInput")
    ones104 = nc.dram_tensor("ones104", [CP, 1], dt.bfloat16, kind="ExternalInput")
    out = nc.dram_tensor("out", [CC, B], dt.float32, kind="ExternalOutput")
    if debug:
        sdbg = nc.dram_tensor("sdbg", [CKC, B], dt.float32, kind="ExternalOutput")
        cdbg = nc.dram_tensor("cdbg", [CP, B], dt.float32, kind="ExternalOutput")

    with tile.TileContext(nc) as tc:
        with (
            tc.tile_pool(name="dram", bufs=1, space="DRAM") as dpool,
            tc.tile_pool(name="consts", bufs=1) as cpool,
            tc.tile_pool(name="chain", bufs=3) as chp,
            tc.tile_pool(name="msb", bufs=1) as mpool,
            tc.tile_pool(name="wt", bufs=1) as wpool,
            tc.tile_pool(name="fb", bufs=1) as fpool,
            tc.tile_pool(name="ep", bufs=1) as epool,
            tc.tile_pool(name="ps", bufs=1, space="PSUM") as psp,
        ):
            # ---------------- constants ----------------
            eye4b_s = cpool.tile([128, 512], dt.bfloat16)
            nc.sync.dma_start(eye4b_s[:], eye4b[:])
            eye1b_s = cpool.tile([128, 128], dt.bfloat16)
            nc.sync.dma_start(eye1b_s[:], eye1b[:])
            oneskt_s = cpool.tile([CKC, CC], dt.bfloat16)
            nc.sync.dma_start(oneskt_s[:], oneskt[:])
            ones104_s = cpool.tile([CP, 1], dt.bfloat16)
            nc.sync.dma_start(ones104_s[:], ones104[:])
            muT_s = cpool.tile([D, CKC], dt.float32)
            nc.sync.dma_start(muT_s[:], muT[:])
            pairmask = nc.dram_tensor("pairmask", [128, CKC], dt.float32,
                                      kind="ExternalInput")
            pairmask_s = cpool.tile([128, CKC], dt.float32)
            nc.sync.dma_start(pairmask_s[:], pairmask[:])
            mu_st = nc.dram_tensor("mu_st", [128, CKC], dt.float32,
                                   kind="ExternalInput")
            mu_st_s = cpool.tile([128, CKC], dt.float32)
            nc.sync.dma_start(mu_st_s[:], mu_st[:])
            ones128f = cpool.tile([128, 1], dt.float32)
            nc.vector.memset(ones128f[:], 1.0)
            ones2_s = cpool.tile([2, B], dt.bfloat16)
            nc.vector.memset(ones2_s[:], 1.0)
            halfones = cpool.tile([128, 2], dt.bfloat16)
            nc.vector.memset(halfones[:], 0.0)
            nc.vector.memset(halfones[0:64, 0:1], 1.0)
            nc.vector.memset(halfones[64:128, 1:2], 1.0)
            neg88 = cpool.tile([CP, 1], dt.float32)
            nc.vector.memset(neg88[:], -88.02969193111305)  # -127*ln2

            LN2 = 0.6931471805599453

            def safe_ln(out_ap, src_ap, pfx):
                # out = ln(src) + 127*ln2, exact for any positive fp32 via
                # exponent/mantissa split (ACT Ln is only good on ~[e-30,e30])
                P, N = src_ap.shape[0], src_ap.shape[-1]
                xb = src_ap.bitcast(dt.int32)
                sh = epool.tile([P, N], dt.int32, tag="slsh", bufs=2,
                                name=f"{pfx}sh")
                nc.vector.tensor_scalar(
                    sh[:], xb, 23, None,
                    op0=mybir.AluOpType.logical_shift_right)
                ef = epool.tile([P, N], dt.float32, tag="slef", bufs=2,
                                name=f"{pfx}ef")
                nc.vector.tensor_copy(ef[:], sh[:])
                mi = epool.tile([P, N], dt.int32, tag="slmi", bufs=2,
                                name=f"{pfx}mi")
                nc.vector.tensor_scalar(
                    mi[:], xb, 0x007FFFFF, 0x3F800000,
                    op0=mybir.AluOpType.bitwise_and,
                    op1=mybir.AluOpType.bitwise_or)
                lnm = epool.tile([P, N], dt.float32, tag="sllnm", bufs=2,
                                 name=f"{pfx}lnm")
                nc.scalar.activation(lnm[:], mi[:].bitcast(dt.float32), AF.Ln)
                nc.vector.scalar_tensor_tensor(
                    out_ap, ef[:], LN2, lnm[:],
                    op0=mybir.AluOpType.mult, op1=mybir.AluOpType.add)

            # -------- phase A: chain -> M (bf16, DRAM ck-major) --------
            Mdram2 = dpool.tile([128, 4096], dt.bfloat16)
            ld_ps = psp.tile([2, NPAIR], dt.float32, tag="aux", bufs=2)
            Mckb = mpool.tile([D, CKC * D], dt.bfloat16)
            muTb = cpool.tile([D, CKC], dt.bfloat16)
            nc.vector.tensor_copy(muTb[:], muT_s[:])
            v2_ps = psp.tile([128, CKC], dt.float32, tag="aux", bufs=2)
            for q in range(NQ):
                qs = slice(512 * q, 512 * q + 512)
                lp_q = chp.tile([128, 512], dt.bfloat16, tag="lp")
                nc.sync.dma_start(lp_q[:], Lp[:, qs])
                lpt_q = chp.tile([128, 512], dt.bfloat16, tag="lpt")
                nc.sync.dma_start(lpt_q[:], LpT[:, qs])
                # logdet contribution: mask out diag, ln, half-partition sums
                eld_q = chp.tile([128, 512], dt.bfloat16, tag="eld")
                nc.vector.tensor_mul(eld_q[:], lp_q[:], eye4b_s[:])
                dg_q = chp.tile([128, 4], dt.float32, tag="dg")
                nc.vector.reduce_sum(
                    dg_q[:], eld_q[:].rearrange("r (p c) -> r p c", c=128),
                    axis=mybir.AxisListType.X)
                dga_q = chp.tile([128, 4], dt.float32, tag="dga")
                nc.scalar.activation(dga_q[:], dg_q[:], AF.Abs)
                dgl_q = chp.tile([128, 4], dt.bfloat16, tag="dgl")
                nc.scalar.activation(dgl_q[:], dga_q[:], AF.Ln)
                nc.tensor.matmul(ld_ps[:, 4 * q:4 * q + 4], halfones[:], dgl_q[:],
                                 start=True, stop=True)
                xb_q = chp.tile([128, 512], dt.bfloat16, tag="xb")
                nc.vector.tensor_sub(xb_q[:], eye4b_s[:], lp_q[:])
                xbt_q = chp.tile([128, 512], dt.bfloat16, tag="xbt")
                nc.vector.tensor_sub(xbt_q[:], eye4b_s[:], lpt_q[:])

                x2_ps = psp.tile([128, 512], dt.float32, tag="big", bufs=4)
                for p in range(4):
                    sl = slice(128 * p, 128 * p + 128)
                    nc.tensor.matmul(x2_ps[:, sl], xbt_q[:, sl], xb_q[:, sl],
                                     start=True, stop=True)
                ix2_q = chp.tile([128, 512], dt.bfloat16, tag="ix2")
                nc.vector.tensor_add(ix2_q[:], x2_ps[:], eye4b_s[:])

                a_ps = psp.tile([128, 512], dt.float32, tag="big", bufs=4)
                for p in range(4):
                    sl = slice(128 * p, 128 * p + 128)
                    nc.tensor.matmul(a_ps[:, sl], eye1b_s[:], ix2_q[:, sl],
                                     start=True, stop=False)
                    nc.tensor.matmul(a_ps[:, sl], xbt_q[:, sl], ix2_q[:, sl],
                                     start=False, stop=True)
                ab_q = chp.tile([128, 512], dt.bfloat16, tag="ab")
                nc.scalar.activation(ab_q[:], a_ps[:], AF.Copy)

                m_ps = psp.tile([128, 512], dt.float32, tag="big", bufs=4)
                for p in range(4):
                    sl = slice(128 * p, 128 * p + 128)
                    nc.tensor.matmul(m_ps[:, sl], ab_q[:, sl], ab_q[:, sl],
                                     start=True, stop=True)
                mb_q = chp.tile([128, 512], dt.bfloat16, tag="mb")
                nc.scalar.activation(mb_q[:], m_ps[:], AF.Copy)
                # write both diag halves to Mdram2[ck, i*64+j]
                md3 = Mdram2[:].rearrange("ck (i j) -> ck i j", j=D)
                for h in range(2):
                    for p in range(4):
                        ck = 8 * q + 2 * p + h
                        nc.sync.dma_start(
                            md3[ck, :, :],
                            mb_q[64 * h:64 * h + 64,
                                 128 * p + 64 * h:128 * p + 64 * h + 64])
                # Mckb slices for this q (base-partition-0 per-ck blocks)
                for h in range(2):
                    dstv = Mckb[:, 512 * q:512 * q + 512].rearrange(
                        "d (p c) -> d p c", c=128)[:, :, 64 * h:64 * h + 64]
                    srcv = mb_q[64 * h:64 * h + 64, :].rearrange(
                        "d (p c) -> d p c", c=128)[:, :, 64 * h:64 * h + 64]
                    nc.sync.dma_start(dstv, srcv)
                # v pair-matmuls for this q
                for p in range(4):
                    pr = 4 * q + p
                    nc.tensor.matmul(v2_ps[:, 2 * pr:2 * pr + 2],
                                     Mckb[:, 128 * pr:128 * pr + 128],
                                     muTb[:, 2 * pr:2 * pr + 2],
                                     start=True, stop=True)

            # -------- phase B: s, c, W tiles --------
            # masked/stacked v (bf16) feeds the main matmul's XR chunk
            v2zb = wpool.tile([128, CKC], dt.bfloat16, tag="v2zb")
            nc.vector.tensor_mul(v2zb[:], v2_ps[:], pairmask_s[:])
            # s = mu . v via elementwise product + ones-matmul (fp32)
            mv2 = epool.tile([128, CKC], dt.float32)
            nc.vector.tensor_mul(mv2[:], v2_ps[:], mu_st_s[:])
            s_ps = psp.tile([1, CKC], dt.float32, tag="aux", bufs=2)
            nc.tensor.matmul(s_ps[:], ones128f[:], mv2[:],
                             start=True, stop=True)

            # logdet accumulated in ld_ps [2, NPAIR] (h, pair)
            logdet_s = epool.tile([2, NPAIR], dt.float32)
            nc.vector.tensor_copy(logdet_s[:], ld_ps[:])

            # logmix = log_softmax_K(mix)
            mix_s = epool.tile([CC, K], dt.float32)
            nc.sync.dma_start(mix_s[:], mixc[:])
            mmax = epool.tile([CC, 1], dt.float32)
            nc.vector.reduce_max(mmax[:], mix_s[:], axis=mybir.AxisListType.X)
            nmmax = epool.tile([CC, 1], dt.float32)
            nc.vector.tensor_scalar_mul(nmmax[:], mmax[:], -1.0)
            mexp = epool.tile([CC, K], dt.float32)
            nc.scalar.activation(mexp[:], mix_s[:], AF.Exp, bias=nmmax[:])
            msum = epool.tile([CC, 1], dt.float32)
            nc.vector.reduce_sum(msum[:], mexp[:], axis=mybir.AxisListType.X)
            mlse = epool.tile([CC, 1], dt.float32)
            nc.scalar.activation(mlse[:], msum[:], AF.Ln)
            lsefull = epool.tile([CC, 1], dt.float32)
            nc.vector.tensor_add(lsefull[:], mmax[:], mlse[:])
            nlse = epool.tile([CC, 1], dt.float32)
            nc.vector.tensor_scalar_mul(nlse[:], lsefull[:], -1.0)
            logmix = epool.tile([CC, K], dt.float32)
            nc.vector.tensor_scalar_add(logmix[:], mix_s[:], nlse[:])

            # fold [NPAIR,2] logdet and [CC,K] logmix into free-dim rows
            # [1, CKC] (order ck = pair*2+h = c*K+k) via a DRAM bounce
            bdr = dpool.tile([CKC, 2], dt.float32)
            bflat = bdr[:].rearrange("ck two -> (ck two)")
            dst_ld = bflat[0::2].rearrange("(p h) -> p h", h=2).transpose([1, 0])
            nc.sync.dma_start(dst_ld, logdet_s[:])
            dst_lm = bflat[1::2].rearrange("(c k) -> c k", k=K)
            nc.sync.dma_start(dst_lm, logmix[:])
            ldrow = epool.tile([1, CKC], dt.float32)
            nc.sync.dma_start(ldrow[:], bdr[:, 0:1].transpose([1, 0]))
            lmrow = epool.tile([1, CKC], dt.float32)
            nc.sync.dma_start(lmrow[:], bdr[:, 1:2].transpose([1, 0]))

            crow = epool.tile([1, CKC], dt.float32)
            nc.vector.scalar_tensor_tensor(
                crow[:], s_ps[:], -0.5, lmrow[:],
                op0=mybir.AluOpType.mult, op1=mybir.AluOpType.add)
            crow2 = epool.tile([1, CKC], dt.float32)
            nc.vector.tensor_sub(crow2[:], crow[:], ldrow[:])
            crow3 = epool.tile([1, CKC], dt.float32)
            nc.vector.tensor_scalar_add(crow3[:], crow2[:],
                                        float(SHIFT - 0.5 * D * LOG2PI))

            # W tiles (bf16) via hardware DMA transpose of Mdram2 slices
            wts = []
            for t in range(NT):
                wt_ = wpool.tile([128, 128], dt.bfloat16, tag=f"wt{t}",
                                 name=f"wt{t}")
                nc.sync.dma_start_transpose(
                    wt_[:], Mdram2[:, 128 * t:128 * t + 128])
                wts.append(wt_)
            c1row = epool.tile([1, CKC], dt.bfloat16)
            nc.vector.tensor_copy(c1row[:], crow3[:])
            crem = epool.tile([1, CKC], dt.float32)
            nc.vector.tensor_sub(crem[:], crow3[:], c1row[:])
            crem_b = epool.tile([1, CKC], dt.bfloat16)
            nc.vector.tensor_copy(crem_b[:], crem[:])
            cbd = dpool.tile([2, CKC], dt.bfloat16)
            nc.sync.dma_start(cbd[0:1, :], c1row[:])
            nc.sync.dma_start(cbd[1:2, :], crem_b[:])
            c2r = wpool.tile([2, CKC], dt.bfloat16, tag="c2r")
            nc.sync.dma_start(c2r[:], cbd[:])

            # -------- phase C: features + main matmul --------
            xr = fpool.tile([128, B], dt.bfloat16, tag="xr")
            nc.sync.dma_start(xr[0:D, :], xt[:])
            nc.sync.dma_start(xr[D:2 * D, :], xt[:])
            xrh = fpool.tile([128, B], dt.bfloat16, tag="xrh")
            nc.vector.tensor_scalar_mul(xrh[:], xr[:], -0.5)

            s_pss = [psp.tile([CKC, 512], dt.float32, tag="big", bufs=4,
                              name=f"spsum{b}") for b in range(NB)]
            for t in range(NT):
                xb_t = fpool.tile([128, B], dt.bfloat16, tag="xb_t", bufs=3,
                                  name=f"xb_t{t}")
                nc.sync.dma_start(xb_t[0:64, :],
                                  xt[2 * t:2 * t + 1, :].broadcast_to([64, B]))
                nc.sync.dma_start(xb_t[64:128, :],
                                  xt[2 * t + 1:2 * t + 2, :].broadcast_to([64, B]))
                f_t = fpool.tile([128, B], dt.bfloat16, tag="f_t", bufs=3,
                                 name=f"f_t{t}")
                nc.vector.tensor_mul(f_t[:], xb_t[:], xrh[:])
                for b in range(NB):
                    nc.tensor.matmul(s_pss[b][:], wts[t][:, 0:CKC],
                                     f_t[:, 512 * b:512 * b + 512],
                                     start=(t == 0), stop=False)
            for b in range(NB):
                nc.tensor.matmul(s_pss[b][:], v2zb[:],
                                 xr[:, 512 * b:512 * b + 512],
                                 start=False, stop=False)
                nc.tensor.matmul(s_pss[b][:], c2r[:],
                                 ones2_s[:, 512 * b:512 * b + 512],
                                 start=False, stop=True)

            # -------- phase D: epilogue (per-b pipelined collectives) --------
            cl_sb = []
            clg_ds = []
            for b in range(NB):
                bs = slice(512 * b, 512 * b + 512)
                e_b = epool.tile([CKC, 512], dt.bfloat16, tag="e_b", bufs=2,
                                 name=f"e_b{b}")
                nc.scalar.activation(e_b[:], s_pss[b][:], AF.Exp)
                ks_ps = psp.tile([CC, 512], dt.float32, tag="ks", bufs=2,
                                 name=f"ksps{b}")
                nc.tensor.matmul(ks_ps[:], oneskt_s[:], e_b[:],
                                 start=True, stop=True)
                cl_b = epool.tile([CC, 512], dt.float32, tag=f"cl{b}",
                                  name=f"cl{b}")
                safe_ln(cl_b[:], ks_ps[:], f"s1{b}")
                cl_sb.append(cl_b)
                if debug:
                    sd = epool.tile([CKC, 512], dt.float32, tag="sd", bufs=2,
                                    name=f"sd{b}")
                    nc.vector.tensor_copy(sd[:], s_pss[b][:])
                    nc.sync.dma_start(sdbg[:, bs], sd[:])
                clpb_d = dpool.tile([CC, 512], dt.float32, name=f"clpd{b}")
                nc.sync.dma_start(clpb_d[:], cl_b[:])
                clgb_d = dpool.tile([CP, 512], dt.float32,
                                    addr_space="Shared", name=f"clgd{b}")
                nc.gpsimd.collective_compute(
                    "AllGather", mybir.AluOpType.bypass,
                    replica_groups=[list(range(NCORES))],
                    ins=[clpb_d[:]], outs=[clgb_d[:]])
                clg_ds.append(clgb_d)

            for b in range(NB):
                bs = slice(512 * b, 512 * b + 512)
                clg_s = epool.tile([CP, 512], dt.float32, tag="clgs", bufs=2,
                                   name=f"clgs{b}")
                nc.sync.dma_start(clg_s[:], clg_ds[b][:])
                if debug:
                    nc.sync.dma_start(cdbg[:, bs], clg_s[:])
                e2_b = epool.tile([CP, 512], dt.bfloat16, tag="e2b", bufs=2,
                                  name=f"e2b{b}")
                nc.scalar.activation(e2_b[:], clg_s[:], AF.Exp, bias=neg88[:])
                cs_ps = psp.tile([1, 512], dt.float32, tag="ks", bufs=2,
                                 name=f"csps{b}")
                nc.tensor.matmul(cs_ps[:], ones104_s[:], e2_b[:],
                                 start=True, stop=True)
                lden = epool.tile([1, 512], dt.float32, tag="lden", bufs=2,
                                  name=f"lden{b}")
                safe_ln(lden[:], cs_ps[:], f"s2{b}")
                ldb_d = dpool.tile([1, 512], dt.float32, name=f"ldbd{b}")
                nc.sync.dma_start(ldb_d[:], lden[:])
                ldb_s = epool.tile([CC, 512], dt.float32, tag="ldbs", bufs=2,
                                   name=f"ldbs{b}")
                nc.sync.dma_start(ldb_s[:], ldb_d[:].broadcast_to([CC, 512]))
                lg_b = epool.tile([CC, 512], dt.float32, tag="lgb", bufs=2,
                                  name=f"lgb{b}")
                nc.vector.tensor_sub(lg_b[:], cl_sb[b][:], ldb_s[:])
                nc.sync.dma_start(out[:, bs], lg_b[:])

    if not nc.is_finalized():
        nc.finalize()
    return nc


def _prep_inputs(representation, mixture_logits, loc, scale_tril):
    import ml_dtypes
    bf16 = ml_dtypes.bfloat16
    f32 = np.float32

    pad = CP - C
    mixp = np.concatenate([np.asarray(mixture_logits, f32),
                           np.zeros((pad, K), f32)], 0)
    locp = np.concatenate([np.asarray(loc, f32),
                           np.full((pad, K, D), PAD_MU, f32)], 0)
    eye = np.eye(D, dtype=f32)
    stp = np.concatenate([np.asarray(scale_tril, f32),
                          np.broadcast_to(eye, (pad, K, D, D)).copy()], 0)

    xtb = np.ascontiguousarray(np.asarray(representation, f32).T).astype(bf16)

    eye4 = np.zeros((128, 512), f32)
    for p in range(4):
        eye4[:, 128 * p:128 * p + 128] = np.eye(128, dtype=f32)
    eye4 = eye4.astype(bf16)
    eye1 = np.eye(128, dtype=f32).astype(bf16)
    onesk = np.zeros((CKC, CC), f32)
    for c in range(CC):
        onesk[K * c:K * c + K, c] = 1.0
    onesk = onesk.astype(bf16)
    ones_cp = np.ones((CP, 1), f32).astype(bf16)

    in_maps = []
    for r in range(NCORES):
        cls = slice(CC * r, CC * r + CC)
        Lck = stp[cls].reshape(CKC, D, D)
        muck = locp[cls].reshape(CKC, D)
        Lpq = np.zeros((NPAIR, 128, 128), f32)
        LpqT = np.zeros((NPAIR, 128, 128), f32)
        for m in range(NPAIR):
            Lpq[m, 0:D, 0:D] = Lck[2 * m]
            Lpq[m, D:2 * D, D:2 * D] = Lck[2 * m + 1]
            LpqT[m, 0:D, 0:D] = Lck[2 * m].T
            LpqT[m, D:2 * D, D:2 * D] = Lck[2 * m + 1].T
        Lp2 = np.ascontiguousarray(Lpq.transpose(1, 0, 2).reshape(128, -1))
        Lp2T = np.ascontiguousarray(LpqT.transpose(1, 0, 2).reshape(128, -1))
        pmask = np.zeros((128, CKC), f32)
        must = np.zeros((128, CKC), f32)
        for ck in range(CKC):
            hh = ck % 2
            pmask[64 * hh:64 * hh + 64, ck] = 1.0
            must[64 * hh:64 * hh + 64, ck] = muck[ck]
        in_maps.append({
            "pairmask": pmask,
            "mu_st": must,
            "Lp": Lp2.astype(bf16),
            "LpT": Lp2T.astype(bf16),
            "xt": xtb,
            "muT": np.ascontiguousarray(muck.T),
            "mixc": np.ascontiguousarray(mixp[cls]),
            "eye4b": eye4,
            "eye1b": eye1,
            "oneskt": onesk,
            "ones104": ones_cp,
        })
    return in_maps


def _postprocess(results):
    rows = [results[r]["out"] for r in range(NCORES)]
    full = np.concatenate(rows, 0)[:C]
    return np.ascontiguousarray(full.T).astype(np.float32)


def kernel(representation, mixture_logits, loc, scale_tril):
    from concourse.bass_utils import run_bass_kernel_spmd
    nc = _build_nc()
    in_maps = _prep_inputs(representation, mixture_logits, loc, scale_tril)
    res = run_bass_kernel_spmd(nc, in_maps, core_ids=list(range(NCORES)))
    return _postprocess(res.results)


# revision 24
# speedup vs baseline: 1.0425x; 1.0425x over previous
"""MASS variational distribution head: MOG class log-likelihood + log_softmax.

Takes FULL inputs, returns FULL output [B, C]. Internally class-sharded
across 8 NeuronCores (13 padded classes per core), single NEFF, one
AllGather of the per-class log-probs before the final log_softmax.

Math per (class c, component k), all on device:
  A = L^{-1}  via truncated doubling A ~= (I+X)(I+X^2), X = I - L
  M = A^T A,  v = M mu,  s = mu^T v,  logdet = sum log|diag L|
  comp(x) = -0.5 x^T M x + v.x - 0.5 s - 0.5 D log(2pi) - logdet + logmix
  class_lp = logsumexp_k comp ; out = log_softmax_c class_lp

comp is evaluated as one feature matmul S = W^T.T @ F over 4224 features
[x_i x_j (4096, -0.5 folded into one x) | x (64) | 1 | 1 | pad], W bf16.
A global SHIFT is folded into the constant so both logsumexps reduce to
plain exp (ScalarE) + ones-matmul sums (TensorE) without max-subtraction.
"""
import functools
import numpy as np

B, D, C, K = 2048, 64, 100, 8
NCORES = 8
CP = 104                 # padded class count (8 * 13)
CC = CP // NCORES        # classes per core = 13
CKC = CC * K             # ck per core = 104
NPAIR = CKC // 2         # 52
NQ = NPAIR // 4          # 13 four-pair batches
NT = D * D // 128        # 32 quad feature chunks
NB = B // 512            # 4 psum column blocks
SHIFT = 100.0
LOG2PI = 1.8378770664093453
PAD_MU = 1.0e3


@functools.lru_cache(maxsize=2)
def _build_nc(debug=False):
    import concourse.bacc as bacc
    import concourse.mybir as mybir
    import concourse.tile as tile

    dt = mybir.dt
    AF = mybir.ActivationFunctionType
    nc = bacc.Bacc("TRN2", target_bir_lowering=False, debug=False,
                   num_devices=NCORES)

    Lp = nc.dram_tensor("Lp", [128, NPAIR * 128], dt.bfloat16, kind="ExternalInput")
    LpT = nc.dram_tensor("LpT", [128, NPAIR * 128], dt.bfloat16, kind="ExternalInput")
    xt = nc.dram_tensor("xt", [D, B], dt.bfloat16, kind="ExternalInput")
    muT = nc.dram_tensor("muT", [D, CKC], dt.float32, kind="ExternalInput")
    mixc = nc.dram_tensor("mixc", [CC, K], dt.float32, kind="ExternalInput")
    eye4b = nc.dram_tensor("eye4b", [128, 512], dt.bfloat16, kind="ExternalInput")
    eye1b = nc.dram_tensor("eye1b", [128, 128], dt.bfloat16, kind="ExternalInput")
    oneskt = nc.dram_tensor("oneskt", [CKC, CC], dt.bfloat16, kind="ExternalInput")
    ones104 = nc.dram_tensor("ones104", [CP, 1], dt.bfloat16, kind="ExternalInput")
    out = nc.dram_tensor("out", [CC, B], dt.float32, kind="ExternalOutput")
    if debug:
        sdbg = nc.dram_tensor("sdbg", [CKC, B], dt.float32, kind="ExternalOutput")
        cdbg = nc.dram_tensor("cdbg", [CP, B], dt.float32, kind="ExternalOutput")

    with tile.TileContext(nc) as tc:
        with (
            tc.tile_pool(name="dram", bufs=1, space="DRAM") as dpool,
            tc.tile_pool(name="consts", bufs=1) as cpool,
            tc.tile_pool(name="chain", bufs=3) as chp,
            tc.tile_pool(name="msb", bufs=1) as mpool,
            tc.tile_pool(name="wt", bufs=1) as wpool,
            tc.tile_pool(name="fb", bufs=1) as fpool,
            tc.tile_pool(name="ep", bufs=1) as epool,
            tc.tile_pool(name="ps", bufs=1, space="PSUM") as psp,
        ):
            # ---------------- constants ----------------
            eye4b_s = cpool.tile([128, 512], dt.bfloat16)
            nc.sync.dma_start(eye4b_s[:], eye4b[:])
            eye1b_s = cpool.tile([128, 128], dt.bfloat16)
            nc.sync.dma_start(eye1b_s[:], eye1b[:])
            oneskt_s = cpool.tile([CKC, CC], dt.bfloat16)
            nc.sync.dma_start(oneskt_s[:], oneskt[:])
            ones104_s = cpool.tile([CP, 1], dt.bfloat16)
            nc.sync.dma_start(ones104_s[:], ones104[:])
            muT_s = cpool.tile([D, CKC], dt.float32)
            nc.sync.dma_start(muT_s[:], muT[:])
            pairmask = nc.dram_tensor("pairmask", [128, CKC], dt.float32,
                                      kind="ExternalInput")
            pairmask_s = cpool.tile([128, CKC], dt.float32)
            nc.sync.dma_start(pairmask_s[:], pairmask[:])
            mu_st = nc.dram_tensor("mu_st", [128, CKC], dt.float32,
                                   kind="ExternalInput")
            mu_st_s = cpool.tile([128, CKC], dt.float32)
            nc.sync.dma_start(mu_st_s[:], mu_st[:])
            ones128f = cpool.tile([128, 1], dt.float32)
            nc.vector.memset(ones128f[:], 1.0)
            ones2_s = cpool.tile([2, B], dt.bfloat16)
            nc.vector.memset(ones2_s[:], 1.0)
            halfones = cpool.tile([128, 2], dt.bfloat16)
            nc.vector.memset(halfones[:], 0.0)
            nc.vector.memset(halfones[0:64, 0:1], 1.0)
            nc.vector.memset(halfones[64:128, 1:2], 1.0)
            neg88 = cpool.tile([CP, 1], dt.float32)
            nc.vector.memset(neg88[:], -88.02969193111305)  # -127*ln2

            LN2 = 0.6931471805599453

            def safe_ln(out_ap, src_ap, pfx):
                # out = ln(src) + 127*ln2, exact for any positive fp32 via
                # exponent/mantissa split (ACT Ln is only good on ~[e-30,e30])
                P, N = src_ap.shape[0], src_ap.shape[-1]
                xb = src_ap.bitcast(dt.int32)
                sh = epool.tile([P, N], dt.int32, tag="slsh", bufs=2,
                                name=f"{pfx}sh")
                nc.vector.tensor_scalar(
                    sh[:], xb, 23, None,
                    op0=mybir.AluOpType.logical_shift_right)
                ef = epool.tile([P, N], dt.float32, tag="slef", bufs=2,
                                name=f"{pfx}ef")
                nc.vector.tensor_copy(ef[:], sh[:])
                mi = epool.tile([P, N], dt.int32, tag="slmi", bufs=2,
                                name=f"{pfx}mi")
                nc.vector.tensor_scalar(
                    mi[:], xb, 0x007FFFFF, 0x3F800000,
                    op0=mybir.AluOpType.bitwise_and,
                    op1=mybir.AluOpType.bitwise_or)
                lnm = epool.tile([P, N], dt.float32, tag="sllnm", bufs=2,
                                 name=f"{pfx}lnm")
                nc.scalar.activation(lnm[:], mi[:].bitcast(dt.float32), AF.Ln)
                nc.vector.scalar_tensor_tensor(
                    out_ap, ef[:], LN2, lnm[:],
                    op0=mybir.AluOpType.mult, op1=mybir.AluOpType.add)

            # -------- phase A: chain -> M (bf16, DRAM ck-major) --------
            Mdram2 = dpool.tile([128, 4096], dt.bfloat16)
            ld_ps = psp.tile([2, NPAIR], dt.float32, tag="aux", bufs=2)
            Mckb = mpool.tile([D, CKC * D], dt.bfloat16)
            muTb = cpool.tile([D, CKC], dt.bfloat16)
            nc.vector.tensor_copy(muTb[:], muT_s[:])
            v2_ps = psp.tile([128, CKC], dt.float32, tag="aux", bufs=2)
            for q in range(NQ):
                qs = slice(512 * q, 512 * q + 512)
                lp_q = chp.tile([128, 512], dt.bfloat16, tag="lp")
                nc.sync.dma_start(lp_q[:], Lp[:, qs])
                lpt_q = chp.tile([128, 512], dt.bfloat16, tag="lpt")
                nc.sync.dma_start(lpt_q[:], LpT[:, qs])
                # logdet contribution: mask out diag, ln, half-partition sums
                eld_q = chp.tile([128, 512], dt.bfloat16, tag="eld")
                nc.vector.tensor_mul(eld_q[:], lp_q[:], eye4b_s[:])
                dg_q = chp.tile([128, 4], dt.float32, tag="dg")
                nc.vector.reduce_sum(
                    dg_q[:], eld_q[:].rearrange("r (p c) -> r p c", c=128),
                    axis=mybir.AxisListType.X)
                dga_q = chp.tile([128, 4], dt.float32, tag="dga")
                nc.scalar.activation(dga_q[:], dg_q[:], AF.Abs)
                dgl_q = chp.tile([128, 4], dt.bfloat16, tag="dgl")
                nc.scalar.activation(dgl_q[:], dga_q[:], AF.Ln)
                nc.tensor.matmul(ld_ps[:, 4 * q:4 * q + 4], halfones[:], dgl_q[:],
                                 start=True, stop=True)
                xb_q = chp.tile([128, 512], dt.bfloat16, tag="xb")
                nc.vector.tensor_sub(xb_q[:], eye4b_s[:], lp_q[:])
                xbt_q = chp.tile([128, 512], dt.bfloat16, tag="xbt")
                nc.vector.tensor_sub(xbt_q[:], eye4b_s[:], lpt_q[:])

                x2_ps = psp.tile([128, 512], dt.float32, tag="big", bufs=4)
                for p in range(4):
                    sl = slice(128 * p, 128 * p + 128)
                    nc.tensor.matmul(x2_ps[:, sl], xbt_q[:, sl], xb_q[:, sl],
                                     start=True, stop=True)
                ix2_q = chp.tile([128, 512], dt.bfloat16, tag="ix2")
                nc.vector.tensor_add(ix2_q[:], x2_ps[:], eye4b_s[:])

                a_ps = psp.tile([128, 512], dt.float32, tag="big", bufs=4)
                for p in range(4):
                    sl = slice(128 * p, 128 * p + 128)
                    nc.tensor.matmul(a_ps[:, sl], eye1b_s[:], ix2_q[:, sl],
                                     start=True, stop=False)
                    nc.tensor.matmul(a_ps[:, sl], xbt_q[:, sl], ix2_q[:, sl],
                                     start=False, stop=True)
                ab_q = chp.tile([128, 512], dt.bfloat16, tag="ab")
                nc.scalar.activation(ab_q[:], a_ps[:], AF.Copy)

                m_ps = psp.tile([128, 512], dt.float32, tag="big", bufs=4)
                for p in range(4):
                    sl = slice(128 * p, 128 * p + 128)
                    nc.tensor.matmul(m_ps[:, sl], ab_q[:, sl], ab_q[:, sl],
                                     start=True, stop=True)
                mb_q = chp.tile([128, 512], dt.bfloat16, tag="mb")
                nc.scalar.activation(mb_q[:], m_ps[:], AF.Copy)
                # write both diag halves to Mdram2[ck, i*64+j]
                md3 = Mdram2[:].rearrange("ck (i j) -> ck i j", j=D)
                for h in range(2):
                    for p in range(4):
                        ck = 8 * q + 2 * p + h
                        nc.sync.dma_start(
                            md3[ck, :, :],
                            mb_q[64 * h:64 * h + 64,
                                 128 * p + 64 * h:128 * p + 64 * h + 64])
                # Mckb slices for this q (base-partition-0 per-ck blocks)
                for h in range(2):
                    dstv = Mckb[:, 512 * q:512 * q + 512].rearrange(
                        "d (p c) -> d p c", c=128)[:, :, 64 * h:64 * h + 64]
                    srcv = mb_q[64 * h:64 * h + 64, :].rearrange(
                        "d (p c) -> d p c", c=128)[:, :, 64 * h:64 * h + 64]
                    nc.sync.dma_start(dstv, srcv)
                # v pair-matmuls for this q
                for p in range(4):
                    pr = 4 * q + p
                    nc.tensor.matmul(v2_ps[:, 2 * pr:2 * pr + 2],
                                     Mckb[:, 128 * pr:128 * pr + 128],
                                     muTb[:, 2 * pr:2 * pr + 2],
                                     start=True, stop=True)

            # -------- phase B: s, c, W tiles --------
            # masked/stacked v (bf16) feeds the main matmul's XR chunk
            v2zb = wpool.tile([128, CKC], dt.bfloat16, tag="v2zb")
            nc.vector.tensor_mul(v2zb[:], v2_ps[:], pairmask_s[:])
            # s = mu . v via elementwise product + ones-matmul (fp32)
            mv2 = epool.tile([128, CKC], dt.float32)
            nc.vector.tensor_mul(mv2[:], v2_ps[:], mu_st_s[:])
            s_ps = psp.tile([1, CKC], dt.float32, tag="aux", bufs=2)
            nc.tensor.matmul(s_ps[:], ones128f[:], mv2[:],
                             start=True, stop=True)

            # logdet accumulated in ld_ps [2, NPAIR] (h, pair)
            logdet_s = epool.tile([2, NPAIR], dt.float32)
            nc.vector.tensor_copy(logdet_s[:], ld_ps[:])

            # logmix = log_softmax_K(mix)
            mix_s = epool.tile([CC, K], dt.float32)
            nc.sync.dma_start(mix_s[:], mixc[:])
            mmax = epool.tile([CC, 1], dt.float32)
            nc.vector.reduce_max(mmax[:], mix_s[:], axis=mybir.AxisListType.X)
            nmmax = epool.tile([CC, 1], dt.float32)
            nc.vector.tensor_scalar_mul(nmmax[:], mmax[:], -1.0)
            mexp = epool.tile([CC, K], dt.float32)
            nc.scalar.activation(mexp[:], mix_s[:], AF.Exp, bias=nmmax[:])
            msum = epool.tile([CC, 1], dt.float32)
            nc.vector.reduce_sum(msum[:], mexp[:], axis=mybir.AxisListType.X)
            mlse = epool.tile([CC, 1], dt.float32)
            nc.scalar.activation(mlse[:], msum[:], AF.Ln)
            lsefull = epool.tile([CC, 1], dt.float32)
            nc.vector.tensor_add(lsefull[:], mmax[:], mlse[:])
            nlse = epool.tile([CC, 1], dt.float32)
            nc.vector.tensor_scalar_mul(nlse[:], lsefull[:], -1.0)
            logmix = epool.tile([CC, K], dt.float32)
            nc.vector.tensor_scalar_add(logmix[:], mix_s[:], nlse[:])

            # fold [NPAIR,2] logdet and [CC,K] logmix into free-dim rows
            # [1, CKC] (order ck = pair*2+h = c*K+k) via a DRAM bounce
            bdr = dpool.tile([CKC, 2], dt.float32)
            bflat = bdr[:].rearrange("ck two -> (ck two)")
            dst_ld = bflat[0::2].rearrange("(p h) -> p h", h=2).transpose([1, 0])
            nc.sync.dma_start(dst_ld, logdet_s[:])
            dst_lm = bflat[1::2].rearrange("(c k) -> c k", k=K)
            nc.sync.dma_start(dst_lm, logmix[:])
            ldrow = epool.tile([1, CKC], dt.float32)
            nc.sync.dma_start(ldrow[:], bdr[:, 0:1].transpose([1, 0]))
            lmrow = epool.tile([1, CKC], dt.float32)
            nc.sync.dma_start(lmrow[:], bdr[:, 1:2].transpose([1, 0]))

            crow = epool.tile([1, CKC], dt.float32)
            nc.vector.scalar_tensor_tensor(
                crow[:], s_ps[:], -0.5, lmrow[:],
                op0=mybir.AluOpType.mult, op1=mybir.AluOpType.add)
            crow2 = epool.tile([1, CKC], dt.float32)
            nc.vector.tensor_sub(crow2[:], crow[:], ldrow[:])
            crow3 = epool.tile([1, CKC], dt.float32)
            nc.vector.tensor_scalar_add(crow3[:], crow2[:],
                                        float(SHIFT - 0.5 * D * LOG2PI))

            # W tiles (bf16) via hardware DMA transpose of Mdram2 slices
            wts = []
            for t in range(NT):
                wt_ = wpool.tile([128, 128], dt.bfloat16, tag=f"wt{t}",
                                 name=f"wt{t}")
                nc.sync.dma_start_transpose(
                    wt_[:], Mdram2[:, 128 * t:128 * t + 128])
                wts.append(wt_)
            c1row = epool.tile([1, CKC], dt.bfloat16)
            nc.vector.tensor_copy(c1row[:], crow3[:])
            crem = epool.tile([1, CKC], dt.float32)
            nc.vector.tensor_sub(crem[:], crow3[:], c1row[:])
            crem_b = epool.tile([1, CKC], dt.bfloat16)
            nc.vector.tensor_copy(crem_b[:], crem[:])
            cbd = dpool.tile([2, CKC], dt.bfloat16)
            nc.sync.dma_start(cbd[0:1, :], c1row[:])
            nc.sync.dma_start(cbd[1:2, :], crem_b[:])
            c2r = wpool.tile([2, CKC], dt.bfloat16, tag="c2r")
            nc.sync.dma_start(c2r[:], cbd[:])

            # -------- phase C: features + main matmul --------
            xr = fpool.tile([128, B], dt.bfloat16, tag="xr")
            nc.sync.dma_start(xr[0:D, :], xt[:])
            nc.sync.dma_start(xr[D:2 * D, :], xt[:])
            xrh = fpool.tile([128, B], dt.bfloat16, tag="xrh")
            nc.vector.tensor_scalar_mul(xrh[:], xr[:], -0.5)

            s_pss = [psp.tile([CKC, 512], dt.float32, tag="big", bufs=4,
                              name=f"spsum{b}") for b in range(NB)]
            # virtual chunk list: 32 quad chunks + XR(v) chunk + const chunk
            chunks = [("q", t) for t in range(NT)] + [("xr", -1), ("c", -1)]
            NGRP = 2
            GSZ = (len(chunks) + NGRP - 1) // NGRP
            fts = {}
            for g in range(NGRP):
                grp = chunks[g * GSZ:(g + 1) * GSZ]
                for kind, t in grp:
                    if kind != "q":
                        continue
                    xb_t = fpool.tile([128, B], dt.bfloat16, tag="xb_t",
                                      bufs=3, name=f"xb_t{t}")
                    nc.sync.dma_start(
                        xb_t[0:64, :],
                        xt[2 * t:2 * t + 1, :].broadcast_to([64, B]))
                    nc.sync.dma_start(
                        xb_t[64:128, :],
                        xt[2 * t + 1:2 * t + 2, :].broadcast_to([64, B]))
                    f_t = fpool.tile([128, B], dt.bfloat16, tag="f_t",
                                     bufs=GSZ + 3, name=f"f_t{t}")
                    nc.vector.tensor_mul(f_t[:], xb_t[:], xrh[:])
                    fts[t] = f_t
                for b in range(NB):
                    bs = slice(512 * b, 512 * b + 512)
                    for ci, (kind, t) in enumerate(grp):
                        first = (g == 0 and ci == 0)
                        last = (g == NGRP - 1 and ci == len(grp) - 1)
                        if kind == "q":
                            lhs, rhs = fts[t][:, bs], wts[t][:, 0:CKC]
                            nc.tensor.matmul(s_pss[b][:], rhs, lhs,
                                             start=first, stop=last)
                        elif kind == "xr":
                            nc.tensor.matmul(s_pss[b][:], v2zb[:], xr[:, bs],
                                             start=first, stop=last)
                        else:
                            nc.tensor.matmul(s_pss[b][:], c2r[:],
                                             ones2_s[:, bs],
                                             start=first, stop=last)

            # ---- phase D: per-b stage-1, partial-denominator AllReduce ----
            cl_sb = []
            cr_ds = []
            for b in range(NB):
                bs = slice(512 * b, 512 * b + 512)
                e_b = epool.tile([CKC, 512], dt.bfloat16, tag="e_b", bufs=2,
                                 name=f"e_b{b}")
                nc.scalar.activation(e_b[:], s_pss[b][:], AF.Exp)
                ks_ps = psp.tile([CC, 512], dt.float32, tag="ks", bufs=2,
                                 name=f"ksps{b}")
                nc.tensor.matmul(ks_ps[:], oneskt_s[:], e_b[:],
                                 start=True, stop=True)
                cl_b = epool.tile([CC, 512], dt.float32, tag=f"cl{b}",
                                  name=f"cl{b}")
                safe_ln(cl_b[:], ks_ps[:], f"s1{b}")
                cl_sb.append(cl_b)
                if debug:
                    sd = epool.tile([CKC, 512], dt.float32, tag="sd", bufs=2,
                                    name=f"sd{b}")
                    nc.vector.tensor_copy(sd[:], s_pss[b][:])
                    nc.sync.dma_start(sdbg[:, bs], sd[:])
                # local partial of the class-softmax denominator
                e2_b = epool.tile([CC, 512], dt.bfloat16, tag="e2b", bufs=2,
                                  name=f"e2b{b}")
                nc.scalar.activation(e2_b[:], cl_b[:], AF.Exp,
                                     bias=neg88[0:CC, :])
                cs_ps = psp.tile([1, 512], dt.float32, tag="ks", bufs=2,
                                 name=f"csps{b}")
                nc.tensor.matmul(cs_ps[:], ones104_s[0:CC, :], e2_b[:],
                                 start=True, stop=True)
                cspart = epool.tile([1, 512], dt.float32, tag="cspart",
                                    bufs=2, name=f"cspart{b}")
                nc.vector.tensor_copy(cspart[:], cs_ps[:])
                crin_d = dpool.tile([1, 512], dt.float32, name=f"crin{b}")
                nc.sync.dma_start(crin_d[:], cspart[:])
                crout_d = dpool.tile([1, 512], dt.float32,
                                     addr_space="Shared", name=f"crout{b}")
                nc.gpsimd.collective_compute(
                    "AllReduce", mybir.AluOpType.add,
                    replica_groups=[list(range(NCORES))],
                    ins=[crin_d[:]], outs=[crout_d[:]])
                cr_ds.append(crout_d)

            for b in range(NB):
                bs = slice(512 * b, 512 * b + 512)
                crs = epool.tile([1, 512], dt.float32, tag="crs", bufs=2,
                                 name=f"crs{b}")
                nc.sync.dma_start(crs[:], cr_ds[b][:])
                lden = epool.tile([1, 512], dt.float32, tag="lden", bufs=2,
                                  name=f"lden{b}")
                safe_ln(lden[:], crs[:], f"s2{b}")
                ldb_d = dpool.tile([1, 512], dt.float32, name=f"ldbd{b}")
                nc.sync.dma_start(ldb_d[:], lden[:])
                ldb_s = epool.tile([CC, 512], dt.float32, tag="ldbs", bufs=2,
                                   name=f"ldbs{b}")
                nc.sync.dma_start(ldb_s[:], ldb_d[:].broadcast_to([CC, 512]))
                lg_b = epool.tile([CC, 512], dt.float32, tag="lgb", bufs=2,
                                  name=f"lgb{b}")
                nc.vector.tensor_sub(lg_b[:], cl_sb[b][:], ldb_s[:])
                nc.sync.dma_start(out[:, bs], lg_b[:])

    if not nc.is_finalized():
        nc.finalize()
    return nc


def _prep_inputs(representation, mixture_logits, loc, scale_tril):
    import ml_dtypes
    bf16 = ml_dtypes.bfloat16
    f32 = np.float32

    pad = CP - C
    mixp = np.concatenate([np.asarray(mixture_logits, f32),
                           np.zeros((pad, K), f32)], 0)
    locp = np.concatenate([np.asarray(loc, f32),
                           np.full((pad, K, D), PAD_MU, f32)], 0)
    eye = np.eye(D, dtype=f32)
    stp = np.concatenate([np.asarray(scale_tril, f32),
                          np.broadcast_to(eye, (pad, K, D, D)).copy()], 0)

    xtb = np.ascontiguousarray(np.asarray(representation, f32).T).astype(bf16)

    eye4 = np.zeros((128, 512), f32)
    for p in range(4):
        eye4[:, 128 * p:128 * p + 128] = np.eye(128, dtype=f32)
    eye4 = eye4.astype(bf16)
    eye1 = np.eye(128, dtype=f32).astype(bf16)
    onesk = np.zeros((CKC, CC), f32)
    for c in range(CC):
        onesk[K * c:K * c + K, c] = 1.0
    onesk = onesk.astype(bf16)
    ones_cp = np.ones((CP, 1), f32).astype(bf16)

    in_maps = []
    for r in range(NCORES):
        cls = slice(CC * r, CC * r + CC)
        Lck = stp[cls].reshape(CKC, D, D)
        muck = locp[cls].reshape(CKC, D)
        Lpq = np.zeros((NPAIR, 128, 128), f32)
        LpqT = np.zeros((NPAIR, 128, 128), f32)
        for m in range(NPAIR):
            Lpq[m, 0:D, 0:D] = Lck[2 * m]
            Lpq[m, D:2 * D, D:2 * D] = Lck[2 * m + 1]
            LpqT[m, 0:D, 0:D] = Lck[2 * m].T
            LpqT[m, D:2 * D, D:2 * D] = Lck[2 * m + 1].T
        Lp2 = np.ascontiguousarray(Lpq.transpose(1, 0, 2).reshape(128, -1))
        Lp2T = np.ascontiguousarray(LpqT.transpose(1, 0, 2).reshape(128, -1))
        pmask = np.zeros((128, CKC), f32)
        must = np.zeros((128, CKC), f32)
        for ck in range(CKC):
            hh = ck % 2
            pmask[64 * hh:64 * hh + 64, ck] = 1.0
            must[64 * hh:64 * hh + 64, ck] = muck[ck]
        in_maps.append({
            "pairmask": pmask,
            "mu_st": must,
            "Lp": Lp2.astype(bf16),
            "LpT": Lp2T.astype(bf16),
            "xt": xtb,
            "muT": np.ascontiguousarray(muck.T),
            "mixc": np.ascontiguousarray(mixp[cls]),
            "eye4b": eye4,
            "eye1b": eye1,
            "oneskt": onesk,
            "ones104": ones_cp,
        })
    return in_maps


def _postprocess(results):
    rows = [results[r]["out"] for r in range(NCORES)]
    full = np.concatenate(rows, 0)[:C]
    return np.ascontiguousarray(full.T).astype(np.float32)


def kernel(representation, mixture_logits, loc, scale_tril):
    from concourse.bass_utils import run_bass_kernel_spmd
    nc = _build_nc()
    in_maps = _prep_inputs(representation, mixture_logits, loc, scale_tril)
    res = run_bass_kernel_spmd(nc, in_maps, core_ids=list(range(NCORES)))
    return _postprocess(res.results)
